# revision 2
# baseline (speedup 1.0000x reference)
"""Trainium2 Bass kernel for nn_MoEModel (conv feature extractor + top-2 MoE).

Strategy (8 NeuronCores):
  - Data-parallel conv trunk: each core runs conv1/pool/conv2/pool on its
    16-image batch shard (fp32, PE-array row/col packing for the small-K
    convolutions).
  - AllGather of flattened features h [128, 12544].
  - Expert-parallel MoE: core e holds expert e's weights [12544, 1000] and
    computes logits for ALL 128 samples (bf16 matmul, fp32 accumulate).
    Gate logits are computed in full fp32 (top-2 selection is numerically
    sensitive).  Each core masks its expert output by its top-2 gate weight
    and a ReduceScatter(+) combines; each core softmaxes its 16-row shard.
"""

import numpy as np

from concourse import bass, bacc, mybir
from concourse.tile import TileContext
from concourse.masks import make_identity
from concourse.bass_utils import run_bass_kernel_spmd

F32 = mybir.dt.float32
BF16 = mybir.dt.bfloat16
AX = mybir.AxisListType
ALU = mybir.AluOpType
ACTF = mybir.ActivationFunctionType

B = 128          # global batch
SH = 16          # batch shard per core
E = 8            # experts == cores
C = 1000         # classes
D = 14 * 14 * 64 # 12544 flattened features
NK = D // 128    # 98 contraction chunks
RG = [list(range(E))]

# W prefetch ring depth (bf16 chunks of [128, 1000])
WBUFS = 40


def _ap(tensor, offset, dims):
    return bass.AP(tensor=tensor, offset=offset, ap=dims)


def build_program():
    nc = bacc.Bacc("TRN2", target_bir_lowering=False, debug=False, num_devices=E)

    # ---- per-core external I/O ----
    x16 = nc.dram_tensor("x16", [SH, 4096], F32, kind="ExternalInput")
    w1 = nc.dram_tensor("w1", [9, 32], F32, kind="ExternalInput")
    b1 = nc.dram_tensor("b1", [128, 1], F32, kind="ExternalInput")
    w2 = nc.dram_tensor("w2", [32, 576], F32, kind="ExternalInput")
    b2 = nc.dram_tensor("b2", [128, 1], F32, kind="ExternalInput")
    gw = nc.dram_tensor("gw", [D, 8], F32, kind="ExternalInput")
    gb128 = nc.dram_tensor("gb128", [128, 8], F32, kind="ExternalInput")
    we = nc.dram_tensor("we", [D, C], F32, kind="ExternalInput")
    be128 = nc.dram_tensor("be128", [128, C], F32, kind="ExternalInput")
    sel = nc.dram_tensor("sel", [128, 8], F32, kind="ExternalInput")
    out16 = nc.dram_tensor("out16", [SH, C], F32, kind="ExternalOutput")

    with TileContext(nc) as tc:
        with (
            tc.tile_pool(name="consts", bufs=1) as cp,
            tc.tile_pool(name="wraw", bufs=3) as wraw,
            tc.tile_pool(name="wbf", bufs=WBUFS) as wbf,
            tc.tile_pool(name="misc", bufs=1) as mp,
            tc.tile_pool(name="dram", bufs=1, space="DRAM") as dp,
        ):
            # ---- constants into SBUF ----
            w1sb = cp.tile([128, 32], F32, tag="w1sb")
            w2sb = cp.tile([128, 576], F32, tag="w2sb")
            for j in range(4):
                nc.gpsimd.dma_start(w1sb[32 * j:32 * j + 9, :], w1[:, :])
                nc.gpsimd.dma_start(w2sb[32 * j:32 * j + 32, :], w2[:, :])
            b1sb = cp.tile([128, 1], F32, tag="b1sb")
            nc.gpsimd.dma_start(b1sb[:], b1[:, :])
            b2sb = cp.tile([128, 1], F32, tag="b2sb")
            nc.gpsimd.dma_start(b2sb[:], b2[:, :])
            idsb = cp.tile([128, 128], F32, tag="idsb")
            make_identity(nc, idsb[:])
            # gate weights laid out [128, 98*8]: col k*8+j = gw[128k+p, j]
            gwsb = cp.tile([128, NK * 8], F32, tag="gwsb")
            nc.gpsimd.dma_start(
                gwsb[:], _ap(gw, 0, [[8, 128], [128 * 8, NK], [1, 8]])
            )
            gbsb = cp.tile([128, 8], F32, tag="gbsb")
            nc.gpsimd.dma_start(gbsb[:], gb128[:, :])
            besb = cp.tile([128, C], F32, tag="besb")
            nc.gpsimd.dma_start(besb[:], be128[:, :])
            selsb = cp.tile([128, 8], F32, tag="selsb")
            nc.gpsimd.dma_start(selsb[:], sel[:, :])

            # ---- DRAM bounce buffers for collectives ----
            h_local = dp.tile([SH, D], F32, tag="h_local")
            h_all = dp.tile([B, D], F32, tag="h_all", addr_space="Shared")
            cc_in = dp.tile([B, C], F32, tag="cc_in")
            cc_out = dp.tile([SH, C], F32, tag="cc_out")

            # ---- expert weight stream: DMA fp32 chunk -> cast to bf16 ring ----
            # (emitted first so the SP queue is dedicated to W streaming; the
            #  Tile scheduler interleaves with everything else by deps)
            wbf_tiles = []
            for k in range(NK):
                wt = wraw.tile([128, C], F32, tag="wt")
                nc.sync.dma_start(wt[:], we[k * 128:(k + 1) * 128, :])
                wb = wbf.tile([128, C], BF16, tag="wb")
                nc.vector.tensor_copy(wb[:], wt[:])
                wbf_tiles.append(wb)

            # =========== conv trunk on the 16-image shard (fp32) ===========
            with (
                tc.tile_pool(name="conv", bufs=1) as cv,
                tc.tile_pool(name="cps", bufs=2, space="PSUM") as cps,
            ):
                for g in range(4):          # 4 groups of 4 images
                    im = cv.tile([128, 3844], F32, tag="im", bufs=2)
                    for j in range(4):
                        img = 4 * g + j
                        for dy in range(3):
                            nc.gpsimd.dma_start(
                                im[32 * j + 3 * dy:32 * j + 3 * dy + 3, :]
                                .rearrange("p (a b) -> p a b", a=62),
                                _ap(x16, img * 4096 + dy * 64,
                                    [[1, 3], [64, 62], [1, 62]]),
                            )
                    c1a = cv.tile([128, 3844], F32, tag="c1a", bufs=1)
                    for t in range(8):      # N tiles over 3844 pixels
                        c0 = t * 512
                        ntile = min(512, 3844 - c0)
                        ps = cps.tile([128, 512], F32, tag="c1ps")
                        for j in range(4):
                            nc.tensor.matmul(
                                ps[32 * j:32 * j + 32, 0:ntile],
                                w1sb[32 * j:32 * j + 9, :],
                                im[32 * j:32 * j + 9, c0:c0 + ntile],
                                start=True, stop=True,
                                tile_position=(32 * j, 32 * j),
                            )
                        nc.scalar.activation(
                            c1a[:, c0:c0 + ntile], ps[:, 0:ntile],
                            ACTF.Relu, bias=b1sb[:], scale=1.0,
                        )
                    # maxpool 2x2: 62x62 -> 31x31
                    m1 = cv.tile([128, 62 * 31], F32, tag="m1", bufs=1)
                    v = c1a[:].rearrange("p (y x) -> p y x", y=62)
                    m1v = m1[:].rearrange("p (y x) -> p y x", y=62)
                    nc.vector.tensor_max(m1v, v[:, :, 0:62:2], v[:, :, 1:62:2])
                    fm1 = cv.tile([128, 961], F32, tag="fm1", bufs=2)
                    m1r = m1[:].rearrange("p (y x) -> p y x", y=62)
                    fm1v = fm1[:].rearrange("p (y x) -> p y x", y=31)
                    nc.vector.tensor_max(
                        fm1v, m1r[:, 0:62:2, :], m1r[:, 1:62:2, :]
                    )

                    # ---- conv2 on this group: out 29x29x64 per image ----
                    fm1y = fm1[:].rearrange("p (y x) -> p y x", y=31)
                    for pair in range(2):
                        c2a = cv.tile([128, 841], F32, tag="c2a", bufs=2)
                        for (r0, nr) in ((0, 17), (17, 12)):
                            ps2 = cps.tile([128, 512], F32, tag="c2ps")
                            for j2 in range(2):
                                jj = 2 * pair + j2
                                for tap in range(9):
                                    dy, dx = tap // 3, tap % 3
                                    rhs = fm1y[32 * jj:32 * jj + 32,
                                               r0 + dy:r0 + dy + nr,
                                               dx:dx + 29]
                                    nc.tensor.matmul(
                                        ps2[64 * j2:64 * j2 + 64, 0:nr * 29],
                                        w2sb[32 * jj:32 * jj + 32,
                                             64 * tap:64 * tap + 64],
                                        rhs,
                                        start=(tap == 0), stop=(tap == 8),
                                        tile_position=(32 * jj, 64 * j2),
                                    )
                            nc.scalar.activation(
                                c2a[:, r0 * 29:(r0 + nr) * 29],
                                ps2[:, 0:nr * 29],
                                ACTF.Relu, bias=b2sb[:], scale=1.0,
                            )
                        # maxpool 2x2: 29x29 -> 14x14 (drop row/col 28)
                        m2 = cv.tile([128, 29 * 14], F32, tag="m2", bufs=1)
                        cv2v = c2a[:].rearrange("p (y x) -> p y x", y=29)
                        m2v = m2[:].rearrange("p (y x) -> p y x", y=29)
                        nc.vector.tensor_max(
                            m2v, cv2v[:, :, 0:28:2], cv2v[:, :, 1:28:2]
                        )
                        fm2 = cv.tile([128, 196], F32, tag="fm2", bufs=2)
                        m2r = m2[:].rearrange("p (y x) -> p y x", y=29)
                        fm2v = fm2[:].rearrange("p (y x) -> p y x", y=14)
                        nc.vector.tensor_max(
                            fm2v, m2r[:, 0:28:2, :], m2r[:, 1:28:2, :]
                        )
                        # transpose [64ch, 196pix] -> h row (pix-major) per image
                        for j2 in range(2):
                            img = 4 * g + 2 * pair + j2
                            hst = cv.tile([98, 128], F32, tag="hst", bufs=2)
                            for half in range(2):
                                pst = cps.tile([98, 64], F32, tag="pst")
                                nc.tensor.transpose(
                                    pst[:],
                                    fm2[64 * j2:64 * j2 + 64,
                                        98 * half:98 * half + 98],
                                    idsb[64 * j2:64 * j2 + 64,
                                         64 * j2:64 * j2 + 64],
                                    tile_position=(64 * j2, 0),
                                )
                                nc.vector.tensor_copy(
                                    hst[:, 64 * half:64 * half + 64], pst[:]
                                )
                            nc.gpsimd.dma_start(
                                _ap(h_local[:].tensor,
                                    h_local[:].offset + img * D,
                                    [[64, 98], [98 * 64, 2], [1, 64]]),
                                hst[:].rearrange("p (h c) -> p h c", h=2),
                            )

            # =========== AllGather h ===========
            nc.gpsimd.collective_compute(
                "AllGather", ALU.bypass, replica_groups=RG,
                ins=[h_local.opt()], outs=[h_all.opt()],
            )

            # =========== gating (fp32) + expert matmul (bf16) ===========
            with (
                tc.tile_pool(name="hload", bufs=4) as hl,
                tc.tile_pool(name="h32p", bufs=3) as h32p,
                tc.tile_pool(name="hbfp", bufs=3) as hbfp,
                tc.tile_pool(name="gp", bufs=1) as gp,
                tc.tile_pool(name="eps", bufs=1, space="PSUM") as epp,
                tc.tile_pool(name="tps", bufs=2, space="PSUM") as tpp,
            ):
                pse_a = epp.tile([128, 512], F32, tag="pse_a")
                pse_b = epp.tile([128, 488], F32, tag="pse_b")
                psg = epp.tile([128, 8], F32, tag="psg")
                for k in range(NK):
                    hb = hl.tile([128, 128], F32, tag="hb")
                    nc.scalar.dma_start(hb[:], h_all[:, k * 128:(k + 1) * 128])
                    pt = tpp.tile([128, 128], F32, tag="pt")
                    nc.tensor.transpose(pt[:], hb[:], idsb[:])
                    h32 = h32p.tile([128, 128], F32, tag="h32")
                    nc.vector.tensor_copy(h32[:], pt[:])
                    hbf = hbfp.tile([128, 128], BF16, tag="hbf")
                    nc.vector.tensor_copy(hbf[:], pt[:])
                    nc.tensor.matmul(
                        psg[:], h32[:], gwsb[:, k * 8:(k + 1) * 8],
                        start=(k == 0), stop=(k == NK - 1),
                    )
                    wb = wbf_tiles[k]
                    nc.tensor.matmul(
                        pse_a[:], hbf[:], wb[:, 0:512],
                        start=(k == 0), stop=(k == NK - 1),
                    )
                    nc.tensor.matmul(
                        pse_b[:], hbf[:], wb[:, 512:C],
                        start=(k == 0), stop=(k == NK - 1),
                    )

                # ---- gate softmax + top-2 mask (all [128, 8] fp32) ----
                g0 = gp.tile([128, 8], F32, tag="g0")
                nc.vector.tensor_add(g0[:], psg[:], gbsb[:])
                gmax = gp.tile([128, 1], F32, tag="gmax")
                nc.vector.reduce_max(gmax[:], g0[:], axis=AX.X)
                gmn = gp.tile([128, 1], F32, tag="gmn")
                nc.vector.tensor_scalar_mul(gmn[:], gmax[:], -1.0)
                gexp = gp.tile([128, 8], F32, tag="gexp")
                gsum = gp.tile([128, 1], F32, tag="gsum")
                nc.scalar.activation(
                    gexp[:], g0[:], ACTF.Exp,
                    bias=gmn[:], scale=1.0, accum_out=gsum[:],
                )
                grec = gp.tile([128, 1], F32, tag="grec")
                nc.vector.reciprocal(grec[:], gsum[:])
                gg = gp.tile([128, 8], F32, tag="gg")
                nc.vector.tensor_scalar_mul(gg[:], gexp[:], grec[:])
                m1t = gp.tile([128, 1], F32, tag="m1t")
                nc.vector.reduce_max(m1t[:], gg[:], axis=AX.X)
                negsel = gp.tile([128, 8], F32, tag="negsel")
                nc.vector.tensor_scalar(
                    negsel[:], gg[:], m1t[:], -2.0,
                    op0=ALU.is_equal, op1=ALU.mult,
                )
                masked = gp.tile([128, 8], F32, tag="masked")
                nc.vector.tensor_add(masked[:], gg[:], negsel[:])
                m2t = gp.tile([128, 1], F32, tag="m2t")
                nc.vector.reduce_max(m2t[:], masked[:], axis=AX.X)
                gsel = gp.tile([128, 8], F32, tag="gsel")
                nc.vector.tensor_mul(gsel[:], gg[:], selsb[:])
                ge = gp.tile([128, 1], F32, tag="ge")
                nc.vector.reduce_sum(ge[:], gsel[:], axis=AX.X)
                selm = gp.tile([128, 1], F32, tag="selm")
                nc.vector.tensor_scalar(
                    selm[:], ge[:], m2t[:], None, op0=ALU.is_ge,
                )
                wsel = gp.tile([128, 1], F32, tag="wsel")
                nc.vector.tensor_mul(wsel[:], ge[:], selm[:])

                # ---- weighted contribution -> ReduceScatter ----
                contrib = gp.tile([128, C], F32, tag="contrib")
                nc.vector.tensor_add(contrib[:, 0:512], pse_a[:], besb[:, 0:512])
                nc.vector.tensor_add(contrib[:, 512:C], pse_b[:], besb[:, 512:C])
                nc.vector.tensor_scalar_mul(contrib[:], contrib[:], wsel[:])
                nc.gpsimd.dma_start(cc_in[:], contrib[:])
                nc.gpsimd.collective_compute(
                    "ReduceScatter", ALU.add, replica_groups=RG,
                    ins=[cc_in.opt()], outs=[cc_out.opt()],
                )

                # ---- final softmax on the 16-row shard ----
                fin = gp.tile([SH, C], F32, tag="fin")
                nc.gpsimd.dma_start(fin[:], cc_out[:])
                fmax = gp.tile([SH, 1], F32, tag="fmax")
                nc.vector.reduce_max(fmax[:], fin[:], axis=AX.X)
                fmn = gp.tile([SH, 1], F32, tag="fmn")
                nc.vector.tensor_scalar_mul(fmn[:], fmax[:], -1.0)
                fexp = gp.tile([SH, C], F32, tag="fexp")
                fsum = gp.tile([SH, 1], F32, tag="fsum")
                nc.scalar.activation(
                    fexp[:], fin[:], ACTF.Exp,
                    bias=fmn[:], scale=1.0, accum_out=fsum[:],
                )
                frec = gp.tile([SH, 1], F32, tag="frec")
                nc.vector.reciprocal(frec[:], fsum[:])
                fout = gp.tile([SH, C], F32, tag="fout")
                nc.vector.tensor_scalar_mul(fout[:], fexp[:], frec[:])
                nc.gpsimd.dma_start(out16[:], fout[:])

    nc.compile()
    return nc


_NC_CACHE = None


def _get_program():
    global _NC_CACHE
    if _NC_CACHE is None:
        _NC_CACHE = build_program()
    return _NC_CACHE


def make_in_maps(x, conv1_w, conv1_b, conv2_w, conv2_b,
                 gate_w, gate_b, expert_w, expert_b):
    x = np.asarray(x, np.float32).reshape(B, 4096)
    w1 = np.ascontiguousarray(np.asarray(conv1_w, np.float32).reshape(9, 32))
    b1 = np.ascontiguousarray(
        np.tile(np.asarray(conv1_b, np.float32), 4).reshape(128, 1))
    w2 = np.ascontiguousarray(
        np.asarray(conv2_w, np.float32).reshape(9, 32, 64)
        .transpose(1, 0, 2).reshape(32, 576))
    b2 = np.ascontiguousarray(
        np.tile(np.asarray(conv2_b, np.float32), 2).reshape(128, 1))
    gw = np.ascontiguousarray(np.asarray(gate_w, np.float32))
    gb128 = np.ascontiguousarray(
        np.broadcast_to(np.asarray(gate_b, np.float32), (128, 8)))
    ew = np.asarray(expert_w, np.float32)
    eb = np.asarray(expert_b, np.float32)
    in_maps = []
    for r in range(E):
        onehot = np.zeros((1, 8), np.float32)
        onehot[0, r] = 1.0
        in_maps.append({
            "x16": np.ascontiguousarray(x[r * SH:(r + 1) * SH]),
            "w1": w1, "b1": b1, "w2": w2, "b2": b2,
            "gw": gw, "gb128": gb128,
            "we": np.ascontiguousarray(ew[r]),
            "be128": np.ascontiguousarray(
                np.broadcast_to(eb[r], (128, C))),
            "sel": np.ascontiguousarray(np.broadcast_to(onehot, (128, 8))),
        })
    return in_maps


def kernel(**inputs):
    nc = _get_program()
    in_maps = make_in_maps(**inputs)
    res = run_bass_kernel_spmd(nc, in_maps, core_ids=list(range(E)))
    return np.concatenate([res.results[r]["out16"] for r in range(E)], axis=0)
